# revision 31
# baseline (speedup 1.0000x reference)
"""Sparse-attention (sliding window 512 + front 256) Trainium2 kernel.

Sharding: 4 sequence groups x 2 head groups over 8 cores. Core c handles
queries [g*1024, (g+1)*1024) with g = c//2 and q-heads [h2*8, h2*8+8) with
h2 = c%2 (kv heads h2*2, h2*2+1). Outputs are bf16 partial y rows; the host
sums the two head-group partials per sequence group (no big f32 downloads).

Layout:
  - scores are computed transposed (sT = kT^T-slot @ qT -> [keys, q]) so the
    exp output feeds the AV matmul directly as the stationary operand; no
    per-tile transposes of attention weights.
  - AV output is natural [q, dv] with an extra ones-column appended to V, so
    column 128 accumulates the softmax denominator for free; normalization is
    one reciprocal + one multiply per q-tile, then a single PE transpose
    feeds the wo matmul.
  - RoPE uses a paired head-dim basis (reals in dims 0..63, imags 64..127)
    via a host-side permutation of wq/wk columns; dot products are unchanged.
  - Sliding-window/front masks are additive f32 inputs applied only where a
    128x128 tile is not fully allowed (plus validity masking for g=0).
"""

import math
import sys

import numpy as np

sys.path.insert(0, "/opt/trn_rl_repo")

import concourse.bass as bass
from concourse import bacc
import concourse.mybir as mybir
import concourse.tile as tile
from concourse.bass_utils import run_bass_kernel_spmd

try:
    import ml_dtypes

    ml_bf16 = ml_dtypes.bfloat16
except ImportError:  # pragma: no cover
    ml_bf16 = np.float32

# Problem constants (hardcoded per contract)
S = 4096
D = 2048
NH = 16
NKV = 4
DQK = 128
DV = 128
WIN = 512
FRONT = 256
THETA = 10000.0
P = 128
KO = D // P  # 16 contraction chunks

NC_ = 8
NG = 4  # sequence groups
NHG = 2  # head groups
HL = 8  # q heads per core
KVL = 2  # kv heads per core
QROWS = 1024  # queries per core
NQT = QROWS // P  # 8 q subtiles
NBAND = 12  # band key tiles per core
NSLOT = NBAND + 2  # + 2 front tiles
KEYS = NSLOT * P  # 1792 key columns

F32 = mybir.dt.float32
BF16 = mybir.dt.bfloat16
NEG = -1.0e9
INV_SQRT_DQK = 1.0 / math.sqrt(DQK)


def _band_lo(s):
    return max(0, s - 4)


def _band_hi(s):
    return min(NQT - 1, s)


def _build_layout():
    """Score-tile emission order, p_all columns, mask columns, av sources.

    Returns (tiles, n_pcols, n_mcols, av_src) where tiles is a list of dicts:
      kind: 'front'|'band', slot: key-tile slot (0..13), q0: first q subtile,
      nt: number of q subtiles, pbase: col offset in p_all,
      masks: list of (tile_off_tiles, width_tiles, mask_col).
    av_src[qs] = list of (p_col, v_slot) of length 7.
    """
    tiles = []
    pcol = 0
    mcol = 0
    # front tiles first so AV accumulation can start early
    for f in range(2):
        for h2 in range(2):
            t = dict(kind="front", slot=NBAND + f, q0=h2 * 4, nt=4, pbase=pcol)
            t["masks"] = [(0, 4, mcol)]
            mcol += 4 * P
            pcol += 4 * P
            tiles.append(t)
    for s in range(NBAND):
        lo, hi = _band_lo(s), _band_hi(s)
        nt = hi - lo + 1
        parts = [(lo, min(nt, 4))]
        if nt > 4:
            parts.append((lo + 4, nt - 4))
        for pi, (q0, pnt) in enumerate(parts):
            t = dict(kind="band", slot=s, q0=q0, nt=pnt, pbase=pcol)
            masks = []
            if s <= 3:
                # anti tile at qs==s plus validity for qs<s (g==0): whole part
                masks.append((0, pnt, mcol))
                mcol += pnt * P
            elif s <= 7:
                if pi == 0:
                    masks.append((0, 1, mcol))  # causal diag at qs == s-4 == q0
                    mcol += P
                else:
                    masks.append((0, 1, mcol))  # anti at qs == s
                    mcol += P
            else:
                masks.append((0, 1, mcol))  # causal diag at qs == s-4 == q0
                mcol += P
            t["masks"] = masks
            pcol += pnt * P
            tiles.append(t)

    av_src = []
    for qs in range(NQT):
        src = []
        for t in tiles:
            if t["kind"] == "front":
                if t["q0"] <= qs < t["q0"] + t["nt"]:
                    src.append((t["pbase"] + (qs - t["q0"]) * P, t["slot"]))
        for t in tiles:
            if t["kind"] == "band" and t["q0"] <= qs < t["q0"] + t["nt"]:
                s = t["slot"]
                if _band_lo(s) <= qs <= _band_hi(s) and qs - 4 <= s:
                    src.append((t["pbase"] + (qs - t["q0"]) * P, s))
        assert len(src) == 7, (qs, len(src))
        av_src.append(src)
    return tiles, pcol, mcol, av_src


SCORE_TILES, N_PCOL, N_MCOL, AV_SRC = _build_layout()


def build_program():
    nc = bacc.Bacc(None, target_bir_lowering=False)

    xa_d = nc.dram_tensor("xa", [P, KO, 4 * P], BF16, kind="ExternalInput")
    xq0_d = nc.dram_tensor("xq0", [P, KO, 4 * P], BF16, kind="ExternalInput")
    xq1_d = nc.dram_tensor("xq1", [P, KO, 4 * P], BF16, kind="ExternalInput")
    xf_d = nc.dram_tensor("xf", [P, KO, 2 * P], BF16, kind="ExternalInput")
    wq_d = nc.dram_tensor("wq", [HL, P, KO * P], BF16, kind="ExternalInput")
    wk_d = nc.dram_tensor("wk", [KVL, P, KO * P], BF16, kind="ExternalInput")
    # wq/wk columns within each head are pre-permuted to the paired RoPE basis
    wv_d = nc.dram_tensor("wv", [P, KO, KVL * P], BF16, kind="ExternalInput")
    wo_d = nc.dram_tensor("wo", [P, HL, D], BF16, kind="ExternalInput")
    cos_d = nc.dram_tensor("cosd", [P, KEYS], BF16, kind="ExternalInput")
    sin_d = nc.dram_tensor("sind", [P, KEYS], BF16, kind="ExternalInput")
    mask_d = nc.dram_tensor("maskd", [P, N_MCOL], BF16, kind="ExternalInput")
    ident_d = nc.dram_tensor("ident", [P, P], BF16, kind="ExternalInput")
    y_d = nc.dram_tensor("y", [NQT, P, D], BF16, kind="ExternalOutput")

    with tile.TileContext(nc) as tc:
        with (
            tc.tile_pool(name="persist", bufs=1) as persist,
            tc.tile_pool(name="bigslot", bufs=1) as bigslot,
            tc.tile_pool(name="wqpool", bufs=2) as wqpool,
            tc.tile_pool(name="qpool", bufs=2) as qpool,
            tc.tile_pool(name="ppool", bufs=2) as ppool,
            tc.tile_pool(name="stage", bufs=2) as stage,
            tc.tile_pool(name="small", bufs=4) as small,
            tc.tile_pool(name="ypool", bufs=2) as ypool,
            tc.tile_pool(name="psP", bufs=4, space="PSUM") as psP,
            tc.tile_pool(name="psA", bufs=2, space="PSUM") as psA,
            tc.tile_pool(name="psT", bufs=2, space="PSUM") as psT,
        ):
            # ---- persistent SBUF ----
            xq0_sb = persist.tile([P, KO, 4 * P], BF16, tag="xq0")
            xq1_sb = persist.tile([P, KO, 4 * P], BF16, tag="xq1")
            xf_sb = persist.tile([P, KO, 2 * P], BF16, tag="xf")
            kT = persist.tile([P, KVL, KEYS], BF16, tag="kT")
            v_sb = persist.tile([P, NSLOT, KVL, 132], BF16, tag="v")
            cos_sb = persist.tile([P, KEYS], BF16, tag="cos")
            sin_sb = persist.tile([P, KEYS], BF16, tag="sin")
            mask_sb = persist.tile([P, N_MCOL], BF16, tag="mask")
            ident = persist.tile([P, P], BF16, tag="id")
            wk_sb = persist.tile([P, KVL, KO * P], BF16, tag="wk")
            wv_sb = persist.tile([P, KO, KVL * P], BF16, tag="wv")
            outT = persist.tile([P, HL, QROWS], BF16, tag="outT")
            # xa shares its slot with wo (wo loads after the kv projections)
            xa_sb = bigslot.tile([P, KO, 4 * P], BF16, tag="xa_wo")

            # x loads on the scalar HWDGE ring, weights/tables on the sync
            # ring, so the first k-proj chunk's inputs land concurrently.
            for kv in range(KVL):
                nc.sync.dma_start(wk_sb[:, kv, : KO * P // 2], wk_d[kv, :, : KO * P // 2])
                nc.sync.dma_start(wk_sb[:, kv, KO * P // 2 :], wk_d[kv, :, KO * P // 2 :])
            nc.scalar.dma_start(xq0_sb[:, : KO // 2, :], xq0_d[:, : KO // 2, :])
            nc.scalar.dma_start(xq0_sb[:, KO // 2 :, :], xq0_d[:, KO // 2 :, :])
            nc.scalar.dma_start(xq1_sb[:, : KO // 2, :], xq1_d[:, : KO // 2, :])
            nc.scalar.dma_start(xq1_sb[:, KO // 2 :, :], xq1_d[:, KO // 2 :, :])
            nc.sync.dma_start(cos_sb[:], cos_d[:])
            nc.sync.dma_start(sin_sb[:], sin_d[:])
            nc.sync.dma_start(wv_sb[:], wv_d[:])
            nc.scalar.dma_start(xa_sb[:, : KO // 2, :], xa_d[:, : KO // 2, :])
            nc.scalar.dma_start(xa_sb[:, KO // 2 :, :], xa_d[:, KO // 2 :, :])
            nc.scalar.dma_start(xf_sb[:], xf_d[:])
            nc.scalar.dma_start(mask_sb[:], mask_d[:])
            nc.sync.dma_start(ident[:], ident_d[:])

            def rope(ps, w, c0, dst):
                """ps: [128, w] f32 psum (paired layout); dst: [128, w] bf16."""
                st = stage.tile([P, 4 * P], BF16, tag="st")
                nc.scalar.copy(st[:, :w], ps[:, :w])
                trc = stage.tile([P, 4 * P], BF16, tag="trc")
                trs = stage.tile([P, 4 * P], BF16, tag="trs")
                nc.vector.tensor_tensor(
                    trc[:, :w], st[:, :w], cos_sb[:, c0 : c0 + w], op=mybir.AluOpType.mult
                )
                nc.vector.tensor_tensor(
                    trs[:, :w], st[:, :w], sin_sb[:, c0 : c0 + w], op=mybir.AluOpType.mult
                )
                his = stage.tile([64, 4 * P], BF16, tag="his")
                hic = stage.tile([64, 4 * P], BF16, tag="hic")
                nc.vector.tensor_copy(his[:, :w], trs[64:128, :w])
                nc.vector.tensor_copy(hic[:, :w], trc[64:128, :w])
                nc.vector.tensor_tensor(
                    dst[0:64], trc[0:64, :w], his[:, :w], op=mybir.AluOpType.subtract
                )
                nc.vector.tensor_tensor(
                    dst[64:128], trs[0:64, :w], hic[:, :w], op=mybir.AluOpType.add
                )

            # ---- Phase 1: k/v projections (+ RoPE on k) ----
            def kproj_chunk(xt, w, c0):
                for kv in range(KVL):
                    ps = psP.tile([P, 4 * P], F32, tag="psP")
                    for ko in range(KO):
                        nc.tensor.matmul(
                            ps[:, :w],
                            wk_sb[:, kv, ko * P : (ko + 1) * P],
                            xt[:, ko, 0:w],
                            start=(ko == 0),
                            stop=(ko == KO - 1),
                        )
                    rope(ps, w, c0, kT[:, kv, c0 : c0 + w])

            def vproj_slots(slots):
                for s in slots:
                    if s < 4:
                        xt, xoff = xa_sb, s * P
                    elif s < 8:
                        xt, xoff = xq0_sb, (s - 4) * P
                    elif s < 12:
                        xt, xoff = xq1_sb, (s - 8) * P
                    else:
                        xt, xoff = xf_sb, (s - 12) * P
                    ps = psP.tile([P, 4 * P], F32, tag="psP")
                    for ko in range(KO):
                        nc.tensor.matmul(
                            ps[:, : KVL * P],
                            xt[:, ko, xoff : xoff + P],
                            wv_sb[:, ko, :],
                            start=(ko == 0),
                            stop=(ko == KO - 1),
                        )
                    for kvi in range(KVL):
                        nc.vector.tensor_copy(
                            v_sb[:, s, kvi, 0:P], ps[:, kvi * P : (kvi + 1) * P]
                        )

            # interleave so each x tensor has compute queued while the next
            # transfer is still in flight
            kproj_chunk(xq0_sb, 512, 512)
            vproj_slots([4, 5, 6, 7])
            kproj_chunk(xq1_sb, 512, 1024)
            vproj_slots([8, 9, 10, 11])
            kproj_chunk(xa_sb, 512, 0)
            vproj_slots([0, 1, 2, 3])
            kproj_chunk(xf_sb, 256, 1536)
            vproj_slots([12, 13])
            nc.vector.memset(v_sb[:, :, :, P : P + 1], 1.0)

            # wo loads into xa's slot once the kv projections are done
            wo_sb = bigslot.tile([P, HL, D], BF16, tag="xa_wo")
            nc.sync.dma_start(wo_sb[:], wo_d[:])

            # ---- Phase 2: per-head q proj + RoPE + attention ----
            # software-pipelined: head hl's AV phase is emitted after head
            # hl+1's projection + score tiles, so the PE always has dense work
            # while the exp/mask chain of the current head drains.
            # tiles needing only qT[:, :512] first: they fill the PE while
            # the second q half's RoPE is still in flight
            tile_order = sorted(
                SCORE_TILES, key=lambda t: 0 if t["q0"] + t["nt"] <= 4 else 1
            )
            prev = None
            for hl in range(HL + 1):
                if hl < HL:
                    kv = hl // 4
                    wq_sb = wqpool.tile([P, KO * P], BF16, tag="wq")
                    nc.scalar.dma_start(wq_sb[:], wq_d[hl])
                    qT = qpool.tile([P, QROWS], BF16, tag="qT")
                    for half in range(2):
                        xh = xq0_sb if half == 0 else xq1_sb
                        ps = psP.tile([P, 4 * P], F32, tag="psP")
                        for ko in range(KO):
                            nc.tensor.matmul(
                                ps[:],
                                wq_sb[:, ko * P : (ko + 1) * P],
                                xh[:, ko, 0:512],
                                start=(ko == 0),
                                stop=(ko == KO - 1),
                            )
                        rope(ps, 512, 512 + half * 512, qT[:, half * 512 : half * 512 + 512])

                    p_all = ppool.tile([P, N_PCOL], BF16, tag="p")
                    for t in tile_order:
                        w = t["nt"] * P
                        ps = psP.tile([P, 4 * P], F32, tag="psP")
                        nc.tensor.matmul(
                            ps[:, :w],
                            kT[:, kv, t["slot"] * P : (t["slot"] + 1) * P],
                            qT[:, t["q0"] * P : (t["q0"] + t["nt"]) * P],
                            start=True,
                            stop=True,
                        )
                        nc.scalar.activation(
                            p_all[:, t["pbase"] : t["pbase"] + w],
                            ps[:, :w],
                            mybir.ActivationFunctionType.Exp,
                            scale=INV_SQRT_DQK,
                        )
                        # 0/1 bf16 masks applied multiplicatively to exp output
                        for off, mw, mc in t["masks"]:
                            sl = slice(t["pbase"] + off * P, t["pbase"] + (off + mw) * P)
                            nc.vector.tensor_tensor(
                                p_all[:, sl],
                                p_all[:, sl],
                                mask_sb[:, mc : mc + mw * P],
                                op=mybir.AluOpType.mult,
                            )

                if prev is not None:
                    hp, pp, kvp = prev
                    for qs in range(NQT):
                        av = psA.tile([P, 132], F32, tag="av")
                        srcs = AV_SRC[qs]
                        for idx, (pc, vs) in enumerate(srcs):
                            nc.tensor.matmul(
                                av[:, : P + 1],
                                pp[:, pc : pc + P],
                                v_sb[:, vs, kvp, 0 : P + 1],
                                start=(idx == 0),
                                stop=(idx == len(srcs) - 1),
                            )
                        r = small.tile([P, 1], F32, tag="r")
                        nc.vector.reciprocal(r[:], av[:, P : P + 1])
                        tmp = small.tile([P, P], BF16, tag="tmp")
                        nc.vector.tensor_tensor(
                            tmp[:], av[:, 0:P], r.to_broadcast((P, P)), op=mybir.AluOpType.mult
                        )
                        pst = psT.tile([P, P], BF16, tag="pst")
                        nc.tensor.transpose(pst[:], tmp[:], ident[:])
                        nc.vector.tensor_copy(outT[:, hp, qs * P : (qs + 1) * P], pst[:])
                if hl < HL:
                    prev = (hl, p_all, kv)

            # ---- Phase 3: y = outT.T @ wo ----
            for qt in range(NQT):
                ysb = ypool.tile([P, D], BF16, tag="y")
                for ng in range(4):
                    yps = psP.tile([P, 4 * P], F32, tag="psP")
                    for hl in range(HL):
                        nc.tensor.matmul(
                            yps[:],
                            outT[:, hl, qt * P : (qt + 1) * P],
                            wo_sb[:, hl, ng * 512 : ng * 512 + 512],
                            start=(hl == 0),
                            stop=(hl == HL - 1),
                        )
                    nc.scalar.copy(ysb[:, ng * 512 : ng * 512 + 512], yps[:])
                    if ng == 1:
                        nc.sync.dma_start(y_d[qt, :, 0:1024], ysb[:, 0:1024])
                nc.sync.dma_start(y_d[qt, :, 1024:2048], ysb[:, 1024:2048])

    return nc


_PROGRAM = None


def _get_program():
    global _PROGRAM
    if _PROGRAM is None:
        _PROGRAM = build_program()
        _PROGRAM.finalize()
    return _PROGRAM


def _rope_tables():
    inv_freq = 1.0 / (THETA ** (np.arange(0, DQK, 2)[: DQK // 2] / DQK))
    t = np.arange(S, dtype=np.float64)
    ang = np.outer(t, inv_freq)  # (S, 64)
    cos_half = np.cos(ang).T.astype(np.float32)  # (64, S)
    sin_half = np.sin(ang).T.astype(np.float32)
    cos_dup = np.concatenate([cos_half, cos_half], 0)  # (128, S)
    sin_dup = np.concatenate([sin_half, sin_half], 0)
    return cos_dup, sin_dup


def _swizzle_x(xslab):
    """[rows, D] f32 -> [P, KO, rows] bf16 with rows as the free dim."""
    r = xslab.shape[0]
    return np.ascontiguousarray(
        xslab.reshape(r, KO, P).transpose(2, 1, 0)
    ).astype(ml_bf16)


def _host_inputs(x, wq, wk, wv, wo):
    x2 = np.asarray(x, np.float32).reshape(S, D)
    perm = np.concatenate([np.arange(0, DQK, 2), np.arange(1, DQK, 2)])
    wq_p = np.asarray(wq, np.float32).reshape(D, NH, DQK)[:, :, perm]
    wk_p = np.asarray(wk, np.float32).reshape(D, NKV, DQK)[:, :, perm]
    wv_r = np.asarray(wv, np.float32).reshape(D, NKV, DV)
    wo_r = np.asarray(wo, np.float32)  # (NH*DV, D)
    cos_dup, sin_dup = _rope_tables()
    ident = np.eye(P, dtype=np.float32).astype(ml_bf16)

    in_maps = []
    for c in range(NC_):
        g, h2 = c // 2, c % 2
        # key positions: 12 band tiles then 2 front tiles
        band0 = g * 8 - 4
        kpos = np.concatenate(
            [np.arange(band0 * P, (band0 + NBAND) * P), np.arange(2 * P)]
        )  # (1792,), may contain negatives for g==0
        valid = kpos >= 0
        kp = np.where(valid, kpos, 0)

        xrows = np.where(valid[:, None], x2[kp], 0.0)  # (1792, D)
        xa = _swizzle_x(xrows[0:512])
        xq0 = _swizzle_x(xrows[512:1024])
        xq1 = _swizzle_x(xrows[1024:1536])
        xf = _swizzle_x(xrows[1536:1792])

        # weights for this core's heads
        h0, kv0 = h2 * HL, h2 * KVL
        wq_c = np.stack(
            [
                np.ascontiguousarray(
                    wq_p[:, h0 + hl, :].reshape(KO, P, P).transpose(1, 0, 2).reshape(P, KO * P)
                )
                for hl in range(HL)
            ]
        ).astype(ml_bf16)
        wk_c = np.stack(
            [
                np.ascontiguousarray(
                    wk_p[:, kv0 + kv, :].reshape(KO, P, P).transpose(1, 0, 2).reshape(P, KO * P)
                )
                for kv in range(KVL)
            ]
        ).astype(ml_bf16)
        wv_c = np.ascontiguousarray(
            wv_r[:, kv0 : kv0 + KVL, :].reshape(KO, P, KVL * P).transpose(1, 0, 2)
        ).astype(ml_bf16)
        wo_c = np.ascontiguousarray(
            wo_r[h0 * DV : (h0 + HL) * DV].reshape(HL, DV, D).transpose(1, 0, 2)
        ).astype(ml_bf16)

        cos_c = np.where(valid[None, :], cos_dup[:, kp], 0.0).astype(ml_bf16)
        sin_c = np.where(valid[None, :], sin_dup[:, kp], 0.0).astype(ml_bf16)

        # 0/1 multiplicative masks (applied to the exp output)
        mask = np.zeros((P, N_MCOL), np.float32)
        qabs0 = g * QROWS
        for t in SCORE_TILES:
            slot = t["slot"]
            if t["kind"] == "front":
                ka = np.arange((slot - NBAND) * P, (slot - NBAND + 1) * P)
            else:
                ka = np.arange((band0 + slot) * P, (band0 + slot + 1) * P)
            for off, mw, mc in t["masks"]:
                qa = qabs0 + (t["q0"] + off) * P + np.arange(mw * P)
                kk = ka[:, None]
                qq = qa[None, :]
                if t["kind"] == "front":
                    ok = (kk < FRONT) & (kk <= qq - WIN)
                else:
                    ok = (kk <= qq) & (kk > qq - WIN) & (kk >= 0)
                mask[:, mc : mc + mw * P] = np.where(ok, 1.0, 0.0)
        mask = mask.astype(ml_bf16)

        in_maps.append(
            {
                "xa": xa.reshape(P, KO, 4 * P),
                "xq0": xq0.reshape(P, KO, 4 * P),
                "xq1": xq1.reshape(P, KO, 4 * P),
                "xf": xf.reshape(P, KO, 2 * P),
                "wq": wq_c,
                "wk": wk_c,
                "wv": wv_c,
                "wo": wo_c,
                "cosd": cos_c,
                "sind": sin_c,
                "maskd": mask,
                "ident": ident,
            }
        )
    return in_maps


def kernel(x, wq, wk, wv, wo, _trace=False, _trace_kwargs=None):
    nc = _get_program()
    in_maps = _host_inputs(x, wq, wk, wv, wo)
    res = run_bass_kernel_spmd(
        nc, in_maps, list(range(NC_)), trace=_trace, **(_trace_kwargs or {})
    )
    y = np.zeros((S, D), np.float32)
    for c in range(NC_):
        g = c // 2
        y[g * QROWS : (g + 1) * QROWS] += np.asarray(
            res.results[c]["y"], ml_bf16
        ).astype(np.float32).reshape(QROWS, D)
    out = y.reshape(1, S, D)
    if _trace:
        return out, res
    return out
